# revision 20
# baseline (speedup 1.0000x reference)
"""GCN block (3 layers) on 8 trn2 NeuronCores, data-parallel over batch.

Math: each layer is X' = (adj + I) @ leaky_relu(X @ W).
Fold each layer's weight into the previous layer's output so every layer
is one big matmul against A_adj = adj (identity handled separately):

    H0 = lrelu(X0 W0)                 (tiny, on-chip)
    G0 = H0 W1 ; Z1 = A G0 + G0 ; H1 = lrelu(Z1)
    G1 = H1 W2 ; Z2 = A G1 + G1 ; H2 = lrelu(Z2)
    G2 = H2     ; X3 = A G2 + G2      (final output)

Per core: 8 samples x 16 features = 128 = partition width. Layouts:
    T-layout  [c=(b,d), m]   (128 partitions, N free)
    N-layout  [m, c]         (m partitions, 128 free)

fp8 trick: adj^T is stored in HBM as fp8e4m3 scaled by 2^12 (entries
are O(1/N)), only 16 MiB -> fully resident in SBUF after a single
layer-0 stream. The big matmuls run fp8 with perf_mode=DoubleRow
(256-row contraction per instruction, ~1.8x PE rate vs fp16). The fp8
error only touches the adjacency term (~2% of output magnitude); the
dominant identity term is added exactly via one fp16 matmul per chunk:
Z^T_chunk += (W_next*4096).T @ H^T_chunk  (G = H W_next, so
G^T = W_next^T H^T). The 2^12 scale divides out in the PSUM->SBUF
combine (folded into the lrelu scales, free).

Schedule: layer 0 is panel-outer (each A panel is consumed for all 8
output chunks as it streams in). Layers 1-2 split the 8 chunks into
phases C(5,6,7) -> A(0,1,2) -> B(3,4) with dedicated PSUM banks per
phase, so each phase's combines + the next layer's tiny matmuls overlap
the following phase's dense matmul stream: PE never idles long enough
for the HAM clock gate to re-throttle, and layer-2 output DMA overlaps
compute (small tail). Tiny G matmuls pack 4 m-tiles per PSUM bank with
a single DVE fp8 copy.
"""

import numpy as np

N_FULL = 4096
D = 16
B_FULL = 64
NCORES = 8
B_CORE = B_FULL // NCORES  # 8
C = B_CORE * D  # 128 partitions
P = 128
NEG_SLOPE = 0.2
SCALE = 4096.0  # 2^12: lifts adj entries (~2/N) into fp8e4m3 normal range
FREE = 512
NCH = N_FULL // FREE        # 8 output column chunks
NPANEL = N_FULL // (2 * P)  # 16 row panels of 256 (DoubleRow pairs)

# chunk -> psum tag; phases emitted in this order for layers 1-2 (the
# last two are single chunks so the layer-2 output tail is tiny)
TAG_OF = {0: "psA", 1: "psA", 2: "psA", 3: "psB", 4: "psB",
          5: "psT", 6: "psT", 7: "psT"}
PHASES = [(5, 6, 7), (0, 1, 2), (3,), (4,)]
# tiny-matmul m-tile groups (4 tiles each), ordered so groups over the
# last-combined chunks come last
GRP_ORDER = [5, 6, 7, 0, 1, 2, 3, 4]

_CACHE = {}


def _leaky(nc, dest, ps, pool, width, scale):
    """dest = leaky_relu(ps*scale) = 0.2*s*ps + relu(0.8*s*ps), PSUM -> SBUF.

    Split across engines: ACT computes t = relu(0.8*s*ps) (positive scale
    commutes with relu), DVE computes dest = ps*(0.2*s) + t. Each
    instruction reads PSUM at most once (HW constraint).
    """
    import concourse.mybir as mybir

    t = pool.tile([P, width], mybir.dt.float32, tag="lk")
    nc.scalar.activation(
        t[:], ps[:], mybir.ActivationFunctionType.Relu,
        scale=(1.0 - NEG_SLOPE) * scale,
    )
    nc.vector.scalar_tensor_tensor(
        dest, ps[:], NEG_SLOPE * scale, t[:], mybir.AluOpType.mult,
        mybir.AluOpType.add,
    )


def _build_nc():
    """Build the Bass module (per-core program)."""
    import concourse.bass as bass
    import concourse.mybir as mybir
    import concourse.tile as tile
    from concourse import bacc

    f32 = mybir.dt.float32
    f16 = mybir.dt.float16
    f8 = mybir.dt.float8e4
    DR = mybir.MatmulPerfMode.DoubleRow
    n = N_FULL

    nc = bacc.Bacc(
        "TRN2", target_bir_lowering=False, debug=False, num_devices=NCORES
    )
    xt_h = nc.dram_tensor("xt", [C, n], f16, kind="ExternalInput")
    # adj^T * 4096 in fp8, panel-major: at[i, p, t, :] = adjT[i*256+t*128+p, :]
    at_h = nc.dram_tensor("at", [NPANEL, P, 2, n], f8, kind="ExternalInput")
    # 7 weights: W0b, W1b, W2b, I128 (tiny G), then W1b*4096, W2b*4096,
    # I*4096 (identity-add matmuls)
    w_h = nc.dram_tensor("wt", [7, P, P], f16, kind="ExternalInput")
    out_h = nc.dram_tensor("out", [C, n], f16, kind="ExternalOutput")

    with tile.TileContext(nc) as tc:
        with (
            tc.tile_pool(name="const", bufs=1) as constp,
            tc.tile_pool(name="xtp", bufs=2) as xtp,
            tc.tile_pool(name="ht", bufs=2) as htp,
            tc.tile_pool(name="g8", bufs=2) as g8p,
            tc.tile_pool(name="outp", bufs=4) as outp,
            tc.tile_pool(name="lk", bufs=2) as lkp,
            tc.tile_pool(name="ps", bufs=1, space="PSUM") as psp,
        ):
            w_sb = constp.tile([P, 7, P], f16)
            nc.sync.dma_start(w_sb[:], w_h[:].rearrange("w p q -> p w q"))

            def ps_chunk(c):
                return psp.tile([P, FREE], f32, tag=TAG_OF[c],
                                bufs=3 if TAG_OF[c] != "psB" else 2,
                                name=f"pc{c}")

            # H0^T = lrelu(W0_blk.T @ X0^T)  (T-layout). x is one 1-MiB
            # DMA on the scalar ring (small chunked DMAs only reach
            # ~85 GB/s and clog the ring ahead of the A panels)
            xt_sb = xtp.tile([C, n], f16, tag="xtc", bufs=1)
            nc.scalar.dma_start(xt_sb[:], xt_h[:])
            ht_cur = htp.tile([C, n], f16)
            for ch in range(NCH):
                ps = ps_chunk(ch)
                nc.tensor.matmul(
                    ps[:], w_sb[:, 0, :],
                    xt_sb[:, ch * FREE:(ch + 1) * FREE],
                    start=True, stop=True,
                )
                _leaky(nc, ht_cur[:, ch * FREE:(ch + 1) * FREE], ps, lkp,
                       FREE, 1.0)

            # A panels: 16 MiB total, resident for all 3 layers. 1-MiB
            # panel DMAs strictly alternating across both HWDGE rings in
            # consumption order (combined measured ~420 GB/s; one ring
            # alone ~290-340, per-DMA completion receipts gap it)
            at_view = [
                constp.tile([P, 2, n], f8, name=f"atc{i}")
                for i in range(NPANEL)
            ]
            for i in range(NPANEL):
                eng = nc.sync if i % 2 == 0 else nc.scalar
                eng.dma_start(at_view[i][:], at_h[i])

            for layer in range(3):
                tiny_idx = 1 + layer   # W1b, W2b, I128
                id_idx = 4 + layer     # W1b*4096, W2b*4096, I*4096
                last = layer == 2

                # tiny: G8[m, c] = ((H^T)^T @ W_blk) quantized to fp8;
                # 4 m-tiles per PSUM bank, one DVE copy per group
                g8 = g8p.tile([P, 2 * NPANEL, P], f8)
                for gi, grp in enumerate(
                    range(NCH) if layer == 0 else GRP_ORDER
                ):
                    psg = psp.tile([P, 4, P], f32, tag="psT", bufs=3,
                                   name="psg")
                    for j in range(4):
                        mt = grp * 4 + j
                        nc.tensor.matmul(
                            psg[:, j, :],
                            ht_cur[:, mt * P:(mt + 1) * P],
                            w_sb[:, tiny_idx, :],
                            start=True,
                            stop=True,
                        )
                    # alternate engines so a copy never queues behind the
                    # previous one and tiny matmuls don't wait on a bank
                    if gi % 2 == 0:
                        nc.vector.tensor_copy(
                            g8[:, grp * 4:(grp + 1) * 4, :], psg[:]
                        )
                    else:
                        nc.scalar.copy(
                            g8[:, grp * 4:(grp + 1) * 4, :], psg[:]
                        )

                def emit_big(chunks, dest, eoff=0):
                    """Identity matmul (start=True, runs during any
                    DMA-wait idle) then DR matmuls over all panels for
                    `chunks`, then combine per chunk."""
                    ps_l = {c: ps_chunk(c) for c in chunks}
                    for c in chunks:
                        nc.tensor.matmul(
                            ps_l[c][:],
                            w_sb[:, id_idx, :],
                            ht_cur[:, c * FREE:(c + 1) * FREE],
                            start=True,
                            stop=False,
                        )
                    for c in chunks:
                        nc.tensor.matmul(
                            ps_l[c][:],
                            g8[:, 0:2, :],
                            at_view[0][:, :, c * FREE:(c + 1) * FREE],
                            start=False,
                            stop=False,
                            perf_mode=DR,
                        )
                    for i in range(1, NPANEL):
                        for c in chunks:
                            nc.tensor.matmul(
                                ps_l[c][:],
                                g8[:, 2 * i:2 * i + 2, :],
                                at_view[i][:, :, c * FREE:(c + 1) * FREE],
                                start=False,
                                stop=(i == NPANEL - 1),
                                perf_mode=DR,
                            )
                    for k, c in enumerate(chunks, start=eoff):
                        if last:
                            oc = outp.tile([C, FREE], f16, tag="oc")
                            if k % 2 == 0:
                                nc.scalar.activation(
                                    oc[:], ps_l[c][:],
                                    mybir.ActivationFunctionType.Copy,
                                    scale=1.0 / SCALE,
                                )
                            else:
                                nc.vector.tensor_scalar_mul(
                                    oc[:], ps_l[c][:], 1.0 / SCALE
                                )
                            oeng = nc.sync if k % 2 == 0 else nc.scalar
                            oeng.dma_start(
                                out_h[:, c * FREE:(c + 1) * FREE], oc[:]
                            )
                        else:
                            _leaky(
                                nc,
                                dest[:, c * FREE:(c + 1) * FREE],
                                ps_l[c],
                                lkp,
                                FREE,
                                1.0 / SCALE,
                            )

                dest = None if last else htp.tile([C, n], f16, name="htn")
                if layer == 0:
                    # panel-outer over all 8 chunks: consume each A panel
                    # for every chunk as it streams in
                    emit_big(tuple(range(NCH)), dest)
                else:
                    for pi, chunks in enumerate(PHASES):
                        emit_big(chunks, dest, eoff=pi)
                ht_cur = dest

    nc.compile()
    return nc


def _get_nc():
    if "nc" not in _CACHE:
        _CACHE["nc"] = _build_nc()
    return _CACHE["nc"]


def _block_diag(w, reps):
    """(D,D) -> (reps*D, reps*D) block diagonal, f32."""
    d = w.shape[0]
    out = np.zeros((reps * d, reps * d), dtype=np.float32)
    for b in range(reps):
        out[b * d:(b + 1) * d, b * d:(b + 1) * d] = w
    return out


def prepare_inputs(x, adj, Identity, W0, W1, W2):
    """Host-side layout prep. Returns per-core input maps."""
    import ml_dtypes

    n = N_FULL
    b_core = B_CORE
    c = C

    # adj^T * 4096 in fp8 (no identity), panel-major for DoubleRow:
    # at8[i, p, t, :] = adjT[i*256 + t*128 + p, :]
    adjt = np.ascontiguousarray(np.asarray(adj, np.float32).T) * SCALE
    at8 = np.ascontiguousarray(
        adjt.reshape(NPANEL, 2, P, n).transpose(0, 2, 1, 3)
    ).astype(ml_dtypes.float8_e4m3)

    reps = c // D
    w0b = _block_diag(np.asarray(W0, np.float32), reps)
    w1b = _block_diag(np.asarray(W1, np.float32), reps)
    w2b = _block_diag(np.asarray(W2, np.float32), reps)
    eye = np.eye(c, dtype=np.float32)
    w_all = np.stack(
        [w0b, w1b, w2b, eye, w1b * SCALE, w2b * SCALE, eye * SCALE]
    ).astype(np.float16)

    # xt[core][b*D+d, m] = x[core*b_core + b, m, d]
    xf = np.asarray(x, np.float32)
    in_maps = []
    for core in range(NCORES):
        xs = xf[core * b_core:(core + 1) * b_core]      # (b_core, n, D)
        xt = np.ascontiguousarray(
            xs.transpose(0, 2, 1).reshape(c, n)
        ).astype(np.float16)
        in_maps.append({"xt": xt, "at": at8, "wt": w_all})
    return in_maps


def gather_output(results, b_full=B_FULL):
    n = N_FULL
    b_core = b_full // NCORES
    c = b_core * D
    out = np.empty((b_full, n, D), dtype=np.float32)
    for core in range(NCORES):
        oc = np.asarray(results[core]["out"]).astype(np.float32)
        oc = oc.reshape(b_core, D, n)
        out[core * b_core:(core + 1) * b_core] = oc.transpose(0, 2, 1)
    return out


def run(x, adj, Identity, W0, W1, W2, trace=False, **_ignored):
    from concourse.bass_utils import run_bass_kernel_spmd

    nc = _get_nc()
    in_maps = prepare_inputs(x, adj, Identity, W0, W1, W2)
    core_ids = list(range(NCORES))
    res = run_bass_kernel_spmd(nc, in_maps, core_ids, trace=trace)
    out = gather_output(res.results, x.shape[0])
    return out, res


def kernel(x, adj, Identity, W0, W1, W2):
    out, _ = run(x, adj, Identity, W0, W1, W2)
    return out


# revision 21
# speedup vs baseline: 1.1075x; 1.1075x over previous
"""GCN block (3 layers) on 8 trn2 NeuronCores, data-parallel over batch.

Math: each layer is X' = (adj + I) @ leaky_relu(X @ W).
Fold each layer's weight into the previous layer's output so every layer
is one big matmul against A_adj = adj (identity handled separately):

    H0 = lrelu(X0 W0)                 (tiny, on-chip)
    G0 = H0 W1 ; Z1 = A G0 + G0 ; H1 = lrelu(Z1)
    G1 = H1 W2 ; Z2 = A G1 + G1 ; H2 = lrelu(Z2)
    G2 = H2     ; X3 = A G2 + G2      (final output)

Per core: 8 samples x 16 features = 128 = partition width. Layouts:
    T-layout  [c=(b,d), m]   (128 partitions, N free)
    N-layout  [m, c]         (m partitions, 128 free)

fp8 trick: adj^T is stored in HBM as fp8e4m3 scaled by 2^12 (entries
are O(1/N)), only 16 MiB -> fully resident in SBUF after a single
layer-0 stream. The big matmuls run fp8 with perf_mode=DoubleRow
(256-row contraction per instruction, ~1.8x PE rate vs fp16). The fp8
error only touches the adjacency term (~2% of output magnitude); the
dominant identity term is added exactly via one fp16 matmul per chunk:
Z^T_chunk += (W_next*4096).T @ H^T_chunk  (G = H W_next, so
G^T = W_next^T H^T). The 2^12 scale divides out in the PSUM->SBUF
combine (folded into the lrelu scales, free).

Schedule: layer 0 is panel-outer (each A panel is consumed for all 8
output chunks as it streams in). Layers 1-2 split the 8 chunks into
phases C(5,6,7) -> A(0,1,2) -> B(3,4) with dedicated PSUM banks per
phase, so each phase's combines + the next layer's tiny matmuls overlap
the following phase's dense matmul stream: PE never idles long enough
for the HAM clock gate to re-throttle, and layer-2 output DMA overlaps
compute (small tail). Tiny G matmuls pack 4 m-tiles per PSUM bank with
a single DVE fp8 copy.
"""

import numpy as np

N_FULL = 4096
D = 16
B_FULL = 64
NCORES = 8
B_CORE = B_FULL // NCORES  # 8
C = B_CORE * D  # 128 partitions
P = 128
NEG_SLOPE = 0.2
SCALE = 4096.0  # 2^12: lifts adj entries (~2/N) into fp8e4m3 normal range
FREE = 512
NCH = N_FULL // FREE        # 8 output column chunks
NPANEL = N_FULL // (2 * P)  # 16 row panels of 256 (DoubleRow pairs)

# chunk -> psum tag; phases emitted in this order for layers 1-2 (the
# last two are single chunks so the layer-2 output tail is tiny)
TAG_OF = {0: "psA", 1: "psA", 2: "psA", 3: "psB", 4: "psB",
          5: "psT", 6: "psT", 7: "psT"}
PHASES = [(5, 6, 7), (0, 1, 2), (3,), (4,)]
# tiny-matmul m-tile groups (4 tiles each), ordered so groups over the
# last-combined chunks come last
GRP_ORDER = [5, 6, 7, 0, 1, 2, 3, 4]

_CACHE = {}


def _leaky(nc, dest, ps, pool, width, scale):
    """dest = leaky_relu(ps*scale) = 0.2*s*ps + relu(0.8*s*ps), PSUM -> SBUF.

    Split across engines: ACT computes t = relu(0.8*s*ps) (positive scale
    commutes with relu), DVE computes dest = ps*(0.2*s) + t. Each
    instruction reads PSUM at most once (HW constraint).
    """
    import concourse.mybir as mybir

    t = pool.tile([P, width], mybir.dt.float32, tag="lk")
    nc.scalar.activation(
        t[:], ps[:], mybir.ActivationFunctionType.Relu,
        scale=(1.0 - NEG_SLOPE) * scale,
    )
    nc.vector.scalar_tensor_tensor(
        dest, ps[:], NEG_SLOPE * scale, t[:], mybir.AluOpType.mult,
        mybir.AluOpType.add,
    )


def _build_nc():
    """Build the Bass module (per-core program)."""
    import concourse.bass as bass
    import concourse.mybir as mybir
    import concourse.tile as tile
    from concourse import bacc

    f32 = mybir.dt.float32
    f16 = mybir.dt.float16
    f8 = mybir.dt.float8e4
    DR = mybir.MatmulPerfMode.DoubleRow
    n = N_FULL

    nc = bacc.Bacc(
        "TRN2", target_bir_lowering=False, debug=False, num_devices=NCORES
    )
    xt_h = nc.dram_tensor("xt", [C, n], f16, kind="ExternalInput")
    # adj^T * 4096 in fp8, panel-major: at[i, p, t, :] = adjT[i*256+t*128+p, :]
    at_h = nc.dram_tensor("at", [NPANEL, P, 2, n], f8, kind="ExternalInput")
    # 7 weights: W0b, W1b, W2b, I128 (tiny G), then W1b*4096, W2b*4096,
    # I*4096 (identity-add matmuls)
    w_h = nc.dram_tensor("wt", [7, P, P], f16, kind="ExternalInput")
    out_h = nc.dram_tensor("out", [C, n], f16, kind="ExternalOutput")

    with tile.TileContext(nc) as tc:
        with (
            tc.tile_pool(name="const", bufs=1) as constp,
            tc.tile_pool(name="xtp", bufs=2) as xtp,
            tc.tile_pool(name="ht", bufs=2) as htp,
            tc.tile_pool(name="g8", bufs=2) as g8p,
            tc.tile_pool(name="outp", bufs=4) as outp,
            tc.tile_pool(name="lk", bufs=2) as lkp,
            tc.tile_pool(name="ps", bufs=1, space="PSUM") as psp,
        ):
            w_sb = constp.tile([P, 7, P], f16)
            nc.sync.dma_start(w_sb[:], w_h[:].rearrange("w p q -> p w q"))

            def ps_chunk(c):
                return psp.tile([P, FREE], f32, tag=TAG_OF[c],
                                bufs=3 if TAG_OF[c] != "psB" else 2,
                                name=f"pc{c}")

            # H0^T = lrelu(W0_blk.T @ X0^T)  (T-layout). x is one 1-MiB
            # DMA on the scalar ring (small chunked DMAs only reach
            # ~85 GB/s and clog the ring ahead of the A panels)
            xt_sb = xtp.tile([C, n], f16, tag="xtc", bufs=1)
            nc.scalar.dma_start(xt_sb[:], xt_h[:])
            ht_cur = htp.tile([C, n], f16)
            for ch in range(NCH):
                ps = ps_chunk(ch)
                nc.tensor.matmul(
                    ps[:], w_sb[:, 0, :],
                    xt_sb[:, ch * FREE:(ch + 1) * FREE],
                    start=True, stop=True,
                )
                _leaky(nc, ht_cur[:, ch * FREE:(ch + 1) * FREE], ps, lkp,
                       FREE, 1.0)

            # A panels: 16 MiB total, resident for all 3 layers. 1-MiB
            # panel DMAs strictly alternating across both HWDGE rings in
            # consumption order (combined measured ~420 GB/s; one ring
            # alone ~290-340, per-DMA completion receipts gap it)
            at_view = [
                constp.tile([P, 2, n], f8, name=f"atc{i}")
                for i in range(NPANEL)
            ]
            for i in range(NPANEL):
                eng = nc.sync if i % 2 == 0 else nc.scalar
                eng.dma_start(at_view[i][:], at_h[i])

            for layer in range(3):
                tiny_idx = 1 + layer   # W1b, W2b, I128
                id_idx = 4 + layer     # W1b*4096, W2b*4096, I*4096
                last = layer == 2

                # tiny: G8[m, c] = ((H^T)^T @ W_blk) quantized to fp8;
                # 4 m-tiles per PSUM bank, one DVE copy per group
                g8 = g8p.tile([P, 2 * NPANEL, P], f8)
                for gi, grp in enumerate(
                    range(NCH) if layer == 0 else GRP_ORDER
                ):
                    psg = psp.tile([P, 4, P], f32, tag="psT", bufs=3,
                                   name="psg")
                    for j in range(4):
                        mt = grp * 4 + j
                        nc.tensor.matmul(
                            psg[:, j, :],
                            ht_cur[:, mt * P:(mt + 1) * P],
                            w_sb[:, tiny_idx, :],
                            start=True,
                            stop=True,
                        )
                    # DVE only: the scalar engine's queue can sit blocked
                    # behind panel-DMA dispatch instructions (HWDGE is
                    # FIFO per engine), which would stall tiny matmuls
                    nc.vector.tensor_copy(
                        g8[:, grp * 4:(grp + 1) * 4, :], psg[:]
                    )

                def emit_big(chunks, dest, eoff=0):
                    """Identity matmul (start=True, runs during any
                    DMA-wait idle) then DR matmuls over all panels for
                    `chunks`, then combine per chunk."""
                    ps_l = {c: ps_chunk(c) for c in chunks}
                    for c in chunks:
                        nc.tensor.matmul(
                            ps_l[c][:],
                            w_sb[:, id_idx, :],
                            ht_cur[:, c * FREE:(c + 1) * FREE],
                            start=True,
                            stop=False,
                        )
                    for c in chunks:
                        nc.tensor.matmul(
                            ps_l[c][:],
                            g8[:, 0:2, :],
                            at_view[0][:, :, c * FREE:(c + 1) * FREE],
                            start=False,
                            stop=False,
                            perf_mode=DR,
                        )
                    for i in range(1, NPANEL):
                        for c in chunks:
                            nc.tensor.matmul(
                                ps_l[c][:],
                                g8[:, 2 * i:2 * i + 2, :],
                                at_view[i][:, :, c * FREE:(c + 1) * FREE],
                                start=False,
                                stop=(i == NPANEL - 1),
                                perf_mode=DR,
                            )
                    for k, c in enumerate(chunks, start=eoff):
                        if last:
                            oc = outp.tile([C, FREE], f16, tag="oc")
                            if k % 2 == 0:
                                nc.scalar.activation(
                                    oc[:], ps_l[c][:],
                                    mybir.ActivationFunctionType.Copy,
                                    scale=1.0 / SCALE,
                                )
                            else:
                                nc.vector.tensor_scalar_mul(
                                    oc[:], ps_l[c][:], 1.0 / SCALE
                                )
                            oeng = nc.sync if k % 2 == 0 else nc.scalar
                            oeng.dma_start(
                                out_h[:, c * FREE:(c + 1) * FREE], oc[:]
                            )
                        else:
                            _leaky(
                                nc,
                                dest[:, c * FREE:(c + 1) * FREE],
                                ps_l[c],
                                lkp,
                                FREE,
                                1.0 / SCALE,
                            )

                dest = None if last else htp.tile([C, n], f16, name="htn")
                if layer == 0:
                    # panel-outer over all 8 chunks: consume each A panel
                    # for every chunk as it streams in
                    emit_big(tuple(range(NCH)), dest)
                else:
                    for pi, chunks in enumerate(PHASES):
                        emit_big(chunks, dest, eoff=pi)
                ht_cur = dest

    nc.compile()
    return nc


def _get_nc():
    if "nc" not in _CACHE:
        _CACHE["nc"] = _build_nc()
    return _CACHE["nc"]


def _block_diag(w, reps):
    """(D,D) -> (reps*D, reps*D) block diagonal, f32."""
    d = w.shape[0]
    out = np.zeros((reps * d, reps * d), dtype=np.float32)
    for b in range(reps):
        out[b * d:(b + 1) * d, b * d:(b + 1) * d] = w
    return out


def prepare_inputs(x, adj, Identity, W0, W1, W2):
    """Host-side layout prep. Returns per-core input maps."""
    import ml_dtypes

    n = N_FULL
    b_core = B_CORE
    c = C

    # adj^T * 4096 in fp8 (no identity), panel-major for DoubleRow:
    # at8[i, p, t, :] = adjT[i*256 + t*128 + p, :]
    adjt = np.ascontiguousarray(np.asarray(adj, np.float32).T) * SCALE
    at8 = np.ascontiguousarray(
        adjt.reshape(NPANEL, 2, P, n).transpose(0, 2, 1, 3)
    ).astype(ml_dtypes.float8_e4m3)

    reps = c // D
    w0b = _block_diag(np.asarray(W0, np.float32), reps)
    w1b = _block_diag(np.asarray(W1, np.float32), reps)
    w2b = _block_diag(np.asarray(W2, np.float32), reps)
    eye = np.eye(c, dtype=np.float32)
    w_all = np.stack(
        [w0b, w1b, w2b, eye, w1b * SCALE, w2b * SCALE, eye * SCALE]
    ).astype(np.float16)

    # xt[core][b*D+d, m] = x[core*b_core + b, m, d]
    xf = np.asarray(x, np.float32)
    in_maps = []
    for core in range(NCORES):
        xs = xf[core * b_core:(core + 1) * b_core]      # (b_core, n, D)
        xt = np.ascontiguousarray(
            xs.transpose(0, 2, 1).reshape(c, n)
        ).astype(np.float16)
        in_maps.append({"xt": xt, "at": at8, "wt": w_all})
    return in_maps


def gather_output(results, b_full=B_FULL):
    n = N_FULL
    b_core = b_full // NCORES
    c = b_core * D
    out = np.empty((b_full, n, D), dtype=np.float32)
    for core in range(NCORES):
        oc = np.asarray(results[core]["out"]).astype(np.float32)
        oc = oc.reshape(b_core, D, n)
        out[core * b_core:(core + 1) * b_core] = oc.transpose(0, 2, 1)
    return out


def run(x, adj, Identity, W0, W1, W2, trace=False, **_ignored):
    from concourse.bass_utils import run_bass_kernel_spmd

    nc = _get_nc()
    in_maps = prepare_inputs(x, adj, Identity, W0, W1, W2)
    core_ids = list(range(NCORES))
    res = run_bass_kernel_spmd(nc, in_maps, core_ids, trace=trace)
    out = gather_output(res.results, x.shape[0])
    return out, res


def kernel(x, adj, Identity, W0, W1, W2):
    out, _ = run(x, adj, Identity, W0, W1, W2)
    return out
